# revision 36
# baseline (speedup 1.0000x reference)
"""Differential cross-attention kernel for Trainium2, 8-core data-parallel
(one batch element per core; full inputs in, full outputs out).

Per core, all matmuls in float32r (full PE rate at N=512, ~tf32 precision):
  qT = (Wq.T/16) @ geneT            [E, NG-chunk]
  kT = Wk.T @ subT                  [E, NS]
  v  = sub @ Wv.T                   [NS, E]
  S_i = q_i k_i^T                   [128m, NS] in PSUM  (i = head 1, 2)
  P_i = exp(S_i), d_i = rowsum      (ACT Exp + accum_out; no max-subtract
                                     needed: logits are ~N(0,1))
  diff = P1/d1 - lam*P2/d2          (DVE; fp32 output to HBM)
  O = diff @ v                      (PE-transposed diff blocks as lhsT)
  out = (O @ (w*(1-l0)*Wo.T)) * rstd   (rstd row-scale commutes with Wo)
  rstd = 1/sqrt(mean(O^2)+eps)      (Quake rsqrt + 2 Newton iters on DVE --
                                     keeps ACT on a single table set, zero
                                     ACT_TABLE_LOADs)

The emission is software-pipelined two subtiles deep so the static per-engine
orders interleave: PE runs S(i) -> ntO(i-2) -> ttDiff(i-1) -> PV(i-1) ->
Wo(i-2); every cross-engine handoff (exp->combine->transpose->cast->matmul)
has >= 1 subtile of slack, which keeps the PE ~93% busy and the HAM clock
warm (2.4 GHz).

Host staging: gene/substructure transposed per batch core, weights
pre-transposed/pre-scaled (attention scale into Wq, rms_weight*(1-l0) into
Wo), lambda_full computed on host from the tiny lambda vectors.
"""
import math

import numpy as np

import concourse.bass as bass
import concourse.mybir as mybir
import concourse.tile as tile
from concourse import bacc
from concourse import bass_utils
from concourse.masks import make_identity

N_CORES = 8
B, NG, NS, E = 8, 4096, 1024, 512
H = E // 2                     # 256, per-head dim
LAMBDA_INIT = 0.8 - 0.6 * math.exp(-0.3 * 0.0)   # depth 0 -> 0.2
RMS_EPS = 1e-5
P = 128                        # partitions
KI = E // P                    # 4 e_in tiles
EO = E // P                    # 4 e_out tiles
NB = NS // P                   # 8 kv tiles
CHUNK = 512                    # m tokens per chunk
NCH = NG // CHUNK              # 8 chunks
JT = CHUNK // P                # 4 m-subtiles per chunk

F32 = mybir.dt.float32
F32R = mybir.dt.float32r
AF = mybir.ActivationFunctionType
ALU = mybir.AluOpType


def _enable_ldw_opt():
    """The default walrus invocation pins --enable-ldw-opt=false, which emits
    a fresh LDWEIGHTS for every matmul; with it on, back-to-back matmuls that
    share a stationary operand skip the reload. Rewrite the flag in the
    compile command (validated: output matches the reference at ~4e-4)."""
    orig = bass_utils.run_command
    if getattr(orig, "_ldw_patched", False):
        return
    def patched(argv, **kw):
        argv = ["--enable-ldw-opt=true" if a == "--enable-ldw-opt=false" else a
                for a in argv]
        return orig(argv, **kw)
    patched._ldw_patched = True
    bass_utils.run_command = patched


def build_kernel():
    _enable_ldw_opt()
    nc = bacc.Bacc("TRN2", target_bir_lowering=False, debug=False,
                   num_devices=N_CORES)
    geneT = nc.dram_tensor("geneT", [E, NG], F32R, kind="ExternalInput").ap()
    subT = nc.dram_tensor("subT", [E, NS], F32R, kind="ExternalInput").ap()
    wqT = nc.dram_tensor("wqT", [E, E], F32R, kind="ExternalInput").ap()
    wkT = nc.dram_tensor("wkT", [E, E], F32R, kind="ExternalInput").ap()
    wvT = nc.dram_tensor("wvT", [E, E], F32R, kind="ExternalInput").ap()
    woT = nc.dram_tensor("woT", [E, E], F32R, kind="ExternalInput").ap()
    lam = nc.dram_tensor("lam", [P, 1], F32, kind="ExternalInput").ap()
    out_d = nc.dram_tensor("out", [NG, E], F32, kind="ExternalOutput").ap()
    diff_d = nc.dram_tensor("diff", [NG, NS], F32R, kind="ExternalOutput").ap()

    with tile.TileContext(nc) as tc:
        emit(tc, geneT, subT, wqT, wkT, wvT, woT, lam, out_d, diff_d)
    nc.compile()
    return nc


def emit(tc, geneT, subT, wqT, wkT, wvT, woT, lam, out_d, diff_d):
    nc = tc.nc
    from contextlib import ExitStack
    with ExitStack() as ctx:
        consts = ctx.enter_context(tc.tile_pool(name="consts", bufs=1))
        kvp = ctx.enter_context(tc.tile_pool(name="kvp", bufs=1))
        gpool = ctx.enter_context(tc.tile_pool(name="gpool", bufs=3))
        qpool = ctx.enter_context(tc.tile_pool(name="qpool", bufs=2))
        ppool = ctx.enter_context(tc.tile_pool(name="ppool", bufs=3))
        dfpool = ctx.enter_context(tc.tile_pool(name="dfpool", bufs=3))
        dTpool = ctx.enter_context(tc.tile_pool(name="dTpool", bufs=3))
        sqpool = ctx.enter_context(tc.tile_pool(name="sqpool", bufs=3))
        nopool = ctx.enter_context(tc.tile_pool(name="nopool", bufs=3))
        ypool = ctx.enter_context(tc.tile_pool(name="ypool", bufs=3))
        dpool = ctx.enter_context(tc.tile_pool(name="dpool", bufs=64))
        # PSUM: 8 banks total = ps_s 2x[128,1024] (4) + ps_u 4x[128,512] (4)
        ps_s = ctx.enter_context(tc.tile_pool(name="ps_s", bufs=2, space="PSUM"))
        ps_u = ctx.enter_context(tc.tile_pool(name="ps_u", bufs=4, space="PSUM"))

        # ---- constants / input staging --------------------------------
        # subT + wk first: the kv-projection matmuls gate everything else,
        # so their inputs must land before the other 5 MB of weights.
        subT_r = subT.rearrange("(ki p) n -> p ki n", p=P)
        subT_sb = kvp.tile([P, KI, NS], F32R, tag="subT")
        w_sb = {}
        for name, wsrc in (("wk", wkT), ("wq", wqT), ("wv", wvT), ("wo", woT)):
            wtile = consts.tile([P, KI, E], F32R, tag=name)
            w_sb[name] = wtile
        # per-ki DMA slices so the first kv/q projections start as soon as
        # their first contraction tile lands instead of after the full 7 MB
        for ki in range(KI):
            nc.sync.dma_start(out=subT_sb[:, ki], in_=subT_r[:, ki])
            nc.sync.dma_start(out=w_sb["wk"][:, ki],
                              in_=wkT.rearrange("(ki p) e -> p ki e", p=P)[:, ki])
        for ki in range(KI):
            nc.sync.dma_start(out=w_sb["wv"][:, ki],
                              in_=wvT.rearrange("(ki p) e -> p ki e", p=P)[:, ki])
        for ki in range(KI):
            nc.sync.dma_start(out=w_sb["wq"][:, ki],
                              in_=wqT.rearrange("(ki p) e -> p ki e", p=P)[:, ki])
        nc.sync.dma_start(out=w_sb["wo"][:],
                          in_=woT.rearrange("(ki p) e -> p ki e", p=P))
        ident_f = consts.tile([P, P], F32, tag="ident_f")
        make_identity(nc, ident_f[:])
        ident = consts.tile([P, P], F32R, tag="ident")
        nc.vector.tensor_copy(ident[:], ident_f[:])
        lam_sb = consts.tile([P, 1], F32, tag="lam")
        nc.sync.dma_start(out=lam_sb[:], in_=lam)

        # ---- kv setup: kT [E, NS] and v [NS, E] ------------------------
        kT_sb = kvp.tile([P, EO, NS], F32R, tag="kT")
        v_sb = kvp.tile([P, NB, E], F32R, tag="v")

        for eo in range(EO):
            for nch in range(NS // 512):
                pk = ps_u.tile([P, 512], F32, tag="u")
                for ki in range(KI):
                    nc.tensor.matmul(
                        pk[:],
                        w_sb["wk"][:, ki, eo * P:(eo + 1) * P],
                        subT_sb[:, ki, nch * 512:(nch + 1) * 512],
                        start=(ki == 0), stop=(ki == KI - 1))
                nc.scalar.copy(kT_sb[:, eo, nch * 512:(nch + 1) * 512], pk[:])

        for nb in range(NB):
            pv = ps_u.tile([P, 512], F32, tag="u")
            for ki in range(KI):
                nc.tensor.matmul(
                    pv[:],
                    subT_sb[:, ki, nb * P:(nb + 1) * P],
                    w_sb["wv"][:, ki, :],
                    start=(ki == 0), stop=(ki == KI - 1))
            nc.scalar.copy(v_sb[:, nb, :], pv[:])

        # ---- main loop over m-chunks ----------------------------------
        def produce_qT(c):
            gT = gpool.tile([P, KI, CHUNK], F32R, tag="gT")
            gsrc = geneT[:, c * CHUNK:(c + 1) * CHUNK].rearrange(
                "(ki p) m -> p ki m", p=P)
            for ki in range(KI):
                nc.sync.dma_start(out=gT[:, ki], in_=gsrc[:, ki])
            qT = qpool.tile([P, EO, CHUNK], F32R, tag="qT")
            for eo in range(EO):
                pq = ps_u.tile([P, 512], F32, tag="u")
                for ki in range(KI):
                    nc.tensor.matmul(
                        pq[:],
                        w_sb["wq"][:, ki, eo * P:(eo + 1) * P],
                        gT[:, ki, :],
                        start=(ki == 0), stop=(ki == KI - 1))
                nc.scalar.copy(qT[:, eo, :], pq[:])
            return qT

        def emit_S_exp(j_in_chunk, qT):
            """S matmuls + exp/accum for one m-subtile; returns softmax state."""
            j = j_in_chunk
            s1 = ps_s.tile([P, NS], F32, tag="s")
            s2 = ps_s.tile([P, NS], F32, tag="s")
            for hk in range(2):
                for nch in range(NS // 512):
                    nc.tensor.matmul(
                        s1[:, nch * 512:(nch + 1) * 512],
                        qT[:, hk, j * P:(j + 1) * P],
                        kT_sb[:, hk, nch * 512:(nch + 1) * 512],
                        start=(hk == 0), stop=(hk == 1))
            p1 = ppool.tile([P, NS], F32, tag="p1")
            d1 = dpool.tile([P, 1], F32, tag="d")
            nc.scalar.activation(out=p1[:], in_=s1[:], func=AF.Exp,
                                 accum_out=d1[:])
            for hk in range(2):
                for nch in range(NS // 512):
                    nc.tensor.matmul(
                        s2[:, nch * 512:(nch + 1) * 512],
                        qT[:, 2 + hk, j * P:(j + 1) * P],
                        kT_sb[:, 2 + hk, nch * 512:(nch + 1) * 512],
                        start=(hk == 0), stop=(hk == 1))
            p2 = ppool.tile([P, NS], F32, tag="p2")
            d2 = dpool.tile([P, 1], F32, tag="d")
            nc.scalar.activation(out=p2[:], in_=s2[:], func=AF.Exp,
                                 accum_out=d2[:])
            return p1, d1, p2, d2

        def emit_softmax_tail(st, m0):
            """reciprocals + combine into diff; DMA diff out."""
            p1, d1, p2, d2 = st
            r1 = dpool.tile([P, 1], F32, tag="d")
            nc.vector.reciprocal(r1[:], d1[:])
            r2 = dpool.tile([P, 1], F32, tag="d")
            nc.vector.reciprocal(r2[:], d2[:])
            nc.vector.tensor_scalar(out=p2[:], in0=p2[:], scalar1=r2[:],
                                    scalar2=lam_sb[:], op0=ALU.mult,
                                    op1=ALU.mult)
            diff = dfpool.tile([P, NS], F32R, tag="diff")
            nc.vector.scalar_tensor_tensor(
                out=diff[:], in0=p1[:], scalar=r1[:], in1=p2[:],
                op0=ALU.mult, op1=ALU.subtract)
            nc.sync.dma_start(out=diff_d[m0:m0 + P, :], in_=diff[:])
            return diff

        def emit_transposes(diff):
            """diff -> diffT via PE transposes, copy to SBUF in quarters so
            the first PV matmuls can start while later blocks transpose."""
            dT = dTpool.tile([P, NS], F32R, tag="dT")
            for half in range(2):
                tt = ps_u.tile([P, 512], F32R, tag="u")
                for q in range(2):
                    for b in range(2):
                        nb = half * 4 + q * 2 + b
                        nc.tensor.transpose(
                            tt[:, (q * 2 + b) * P:(q * 2 + b + 1) * P],
                            diff[:, nb * P:(nb + 1) * P], ident[:])
                    nc.vector.tensor_copy(
                        dT[:, (half * 2 + q) * 256:(half * 2 + q + 1) * 256],
                        tt[:, q * 256:(q + 1) * 256])
            return dT

        def emit_back(dT, m0):
            """PV + RMS + Wo projection + store for one m-subtile."""
            po = ps_u.tile([P, E], F32, tag="u")
            for nb in range(NB):
                nc.tensor.matmul(
                    po[:],
                    dT[:, nb * P:(nb + 1) * P],
                    v_sb[:, nb, :],
                    start=(nb == 0), stop=(nb == NB - 1))

            # RMS statistic (squares + row-sum in one DVE pass)
            o_sb = nopool.tile([P, E], F32R, tag="no")
            nc.scalar.copy(o_sb[:], po[:])
            sq = sqpool.tile([P, E], F32, tag="sq")
            ssq = dpool.tile([P, 1], F32, tag="d")
            nc.vector.scalar_tensor_tensor(
                out=sq[:], in0=o_sb[:], scalar=1.0, in1=o_sb[:],
                op0=ALU.mult, op1=ALU.mult, accum_out=ssq[:])
            t_ssq = dpool.tile([P, 1], F32, tag="d")
            nc.vector.tensor_scalar(out=t_ssq[:], in0=ssq[:],
                                    scalar1=1.0 / E, scalar2=RMS_EPS,
                                    op0=ALU.mult, op1=ALU.add)
            # rstd = 1/sqrt(t): Quake bit-trick + 2 Newton iterations on DVE
            I32 = mybir.dt.int32
            ihalf = dpool.tile([P, 1], F32, tag="d")
            nc.vector.tensor_scalar(
                out=ihalf[:].bitcast(I32), in0=t_ssq[:].bitcast(I32),
                scalar1=1, scalar2=None, op0=ALU.arith_shift_right)
            inot = dpool.tile([P, 1], F32, tag="d")
            nc.vector.tensor_scalar(
                out=inot[:].bitcast(I32), in0=ihalf[:].bitcast(I32),
                scalar1=-1, scalar2=None, op0=ALU.bitwise_xor)
            yq = dpool.tile([P, 1], F32, tag="d")
            nc.vector.tensor_scalar(
                out=yq[:].bitcast(I32), in0=inot[:].bitcast(I32),
                scalar1=0x5f3759df + 1, scalar2=None, op0=ALU.add)
            rstd = yq
            for _ in range(2):
                y2 = dpool.tile([P, 1], F32, tag="d")
                nc.vector.tensor_mul(y2[:], rstd[:], rstd[:])
                w = dpool.tile([P, 1], F32, tag="d")
                nc.vector.tensor_scalar(out=w[:], in0=y2[:],
                                        scalar1=t_ssq[:], scalar2=-0.5,
                                        op0=ALU.mult, op1=ALU.mult)
                yn = dpool.tile([P, 1], F32, tag="d")
                nc.vector.scalar_tensor_tensor(
                    out=yn[:], in0=w[:], scalar=1.5, in1=rstd[:],
                    op0=ALU.add, op1=ALU.mult)
                rstd = yn

            return o_sb, rstd

        def emit_nt(o_sb):
            """transpose O (grouped with the diff transposes of the next
            subtile so PE transpose-mode switches stay rare)."""
            nt = ps_u.tile([P, E], F32R, tag="u")
            for eb in range(EO):
                nc.tensor.transpose(nt[:, eb * P:(eb + 1) * P],
                                    o_sb[:, eb * P:(eb + 1) * P], ident[:])
            noT = nopool.tile([P, E], F32R, tag="noT")
            nc.scalar.copy(noT[:, :256], nt[:, :256])
            nc.scalar.copy(noT[:, 256:], nt[:, 256:])
            return noT

        def emit_y(noT, rstd, m0):
            py = ps_u.tile([P, E], F32, tag="u")
            for eb in range(EO):
                nc.tensor.matmul(
                    py[:],
                    noT[:, eb * P:(eb + 1) * P],
                    w_sb["wo"][:, eb, :],
                    start=(eb == 0), stop=(eb == EO - 1))
            y = ypool.tile([P, E], F32, tag="y")
            nc.scalar.activation(out=y[:], in_=py[:], func=AF.Copy,
                                 scale=rstd[:])
            nc.sync.dma_start(out=out_d[m0:m0 + P, :], in_=y[:])

        # Software-pipelined emission, three-deep skew. Per step the PE
        # stream is: S(i) -> nt(i-3) -> tt(i-1) -> PV(i-2) -> y(i-3).
        # Transposes of consecutive stages sit adjacent (fewer PE
        # transpose-mode switches) and every cross-engine hop -- including
        # the DVE diffT-cast feeding PV -- has a full subtile of slack.
        NTOT = NCH * JT
        cur_qT, next_qT = produce_qT(0), None
        diffs, dTs, pvs, noTs = {}, {}, {}, {}
        for idx in range(NTOT + 3):
            c, j = divmod(idx, JT)
            if idx < NTOT:
                if j == 0 and c > 0:
                    cur_qT = next_qT
                st = emit_S_exp(j, cur_qT)
            if idx - 3 >= 0:
                noTs[idx - 3] = emit_nt(pvs[idx - 3][0])
            if idx - 1 >= 0 and idx - 1 < NTOT:
                dTs[idx - 1] = emit_transposes(diffs.pop(idx - 1))
            if idx < NTOT:
                diffs[idx] = emit_softmax_tail(st, idx * P)
            if idx - 2 >= 0 and idx - 2 < NTOT:
                pvs[idx - 2] = emit_back(dTs.pop(idx - 2), (idx - 2) * P)
            if idx - 3 >= 0:
                emit_y(noTs.pop(idx - 3), pvs.pop(idx - 3)[1], (idx - 3) * P)
            if idx < NTOT and j == 0 and c + 1 < NCH:
                next_qT = produce_qT(c + 1)


# ---------------------------------------------------------------------------
_NC = None


def get_nc():
    global _NC
    if _NC is None:
        _NC = build_kernel()
    return _NC


def stage_inputs(gene, substructure, Wq, Wk, Wv, Wo,
                 lambda_q1, lambda_k1, lambda_q2, lambda_k2, rms_weight):
    gene = np.asarray(gene, np.float32)
    substructure = np.asarray(substructure, np.float32)
    scaling = H ** -0.5
    lam_full = (math.exp(float(np.sum(np.asarray(lambda_q1, np.float64) *
                                      np.asarray(lambda_k1, np.float64))))
                - math.exp(float(np.sum(np.asarray(lambda_q2, np.float64) *
                                        np.asarray(lambda_k2, np.float64))))
                + LAMBDA_INIT)
    wqT = np.ascontiguousarray(np.asarray(Wq, np.float32).T * scaling)
    wkT = np.ascontiguousarray(np.asarray(Wk, np.float32).T)
    wvT = np.ascontiguousarray(np.asarray(Wv, np.float32).T)
    woT = np.ascontiguousarray(
        np.asarray(rms_weight, np.float32)[:, None]
        * np.asarray(Wo, np.float32).T * (1.0 - LAMBDA_INIT))
    lam_tile = np.full((P, 1), lam_full, np.float32)
    in_maps = []
    for b in range(N_CORES):
        in_maps.append({
            "geneT": np.ascontiguousarray(gene[b].T),
            "subT": np.ascontiguousarray(substructure[b].T),
            "wqT": wqT, "wkT": wkT, "wvT": wvT, "woT": woT,
            "lam": lam_tile,
        })
    return in_maps


def run(in_maps, trace=False, **kw):
    nc = get_nc()
    last_err = None
    for attempt in range(3):
        try:
            return bass_utils.run_bass_kernel_spmd(
                nc, in_maps, core_ids=list(range(N_CORES)), trace=trace, **kw)
        except Exception as e:  # transient device errors on first touch
            last_err = e
    raise last_err


def kernel(**inputs):
    in_maps = stage_inputs(**inputs)
    res = run(in_maps, trace=False)
    out = np.stack([res.results[b]["out"] for b in range(N_CORES)])
    diff = np.stack([res.results[b]["diff"] for b in range(N_CORES)])
    return out, diff


# revision 38
# speedup vs baseline: 1.0133x; 1.0133x over previous
"""Differential cross-attention kernel for Trainium2, 8-core data-parallel
(one batch element per core; full inputs in, full outputs out).

Per core, all matmuls in float32r (full PE rate at N=512, ~tf32 precision):
  qT = (Wq.T/16) @ geneT            [E, NG-chunk]
  kT = Wk.T @ subT                  [E, NS]
  v  = sub @ Wv.T                   [NS, E]
  S_i = q_i k_i^T                   [128m, NS] in PSUM  (i = head 1, 2)
  P_i = exp(S_i), d_i = rowsum      (ACT Exp + accum_out; no max-subtract
                                     needed: logits are ~N(0,1))
  diff = P1/d1 - lam*P2/d2          (DVE; fp32 output to HBM)
  O = diff @ v                      (PE-transposed diff blocks as lhsT)
  out = (O @ (w*(1-l0)*Wo.T)) * rstd   (rstd row-scale commutes with Wo)
  rstd = 1/sqrt(mean(O^2)+eps)      (Quake rsqrt + 2 Newton iters on DVE --
                                     keeps ACT on a single table set, zero
                                     ACT_TABLE_LOADs)

The emission is software-pipelined two subtiles deep so the static per-engine
orders interleave: PE runs S(i) -> ntO(i-2) -> ttDiff(i-1) -> PV(i-1) ->
Wo(i-2); every cross-engine handoff (exp->combine->transpose->cast->matmul)
has >= 1 subtile of slack, which keeps the PE ~93% busy and the HAM clock
warm (2.4 GHz).

Host staging: gene/substructure transposed per batch core, weights
pre-transposed/pre-scaled (attention scale into Wq, rms_weight*(1-l0) into
Wo), lambda_full computed on host from the tiny lambda vectors.
"""
import math

import numpy as np

import concourse.bass as bass
import concourse.mybir as mybir
import concourse.tile as tile
from concourse import bacc
from concourse import bass_utils
from concourse.masks import make_identity

N_CORES = 8
B, NG, NS, E = 8, 4096, 1024, 512
H = E // 2                     # 256, per-head dim
LAMBDA_INIT = 0.8 - 0.6 * math.exp(-0.3 * 0.0)   # depth 0 -> 0.2
RMS_EPS = 1e-5
P = 128                        # partitions
KI = E // P                    # 4 e_in tiles
EO = E // P                    # 4 e_out tiles
NB = NS // P                   # 8 kv tiles
CHUNK = 512                    # m tokens per chunk
NCH = NG // CHUNK              # 8 chunks
JT = CHUNK // P                # 4 m-subtiles per chunk

F32 = mybir.dt.float32
F32R = mybir.dt.float32r
AF = mybir.ActivationFunctionType
ALU = mybir.AluOpType


def build_kernel():
    nc = bacc.Bacc("TRN2", target_bir_lowering=False, debug=False,
                   num_devices=N_CORES)
    geneT = nc.dram_tensor("geneT", [E, NG], F32R, kind="ExternalInput").ap()
    subT = nc.dram_tensor("subT", [E, NS], F32R, kind="ExternalInput").ap()
    wqT = nc.dram_tensor("wqT", [E, E], F32R, kind="ExternalInput").ap()
    wkT = nc.dram_tensor("wkT", [E, E], F32R, kind="ExternalInput").ap()
    wvT = nc.dram_tensor("wvT", [E, E], F32R, kind="ExternalInput").ap()
    woT = nc.dram_tensor("woT", [E, E], F32R, kind="ExternalInput").ap()
    lam = nc.dram_tensor("lam", [P, 1], F32, kind="ExternalInput").ap()
    out_d = nc.dram_tensor("out", [NG, E], F32, kind="ExternalOutput").ap()
    diff_d = nc.dram_tensor("diff", [NG, NS], F32R, kind="ExternalOutput").ap()

    with tile.TileContext(nc) as tc:
        emit(tc, geneT, subT, wqT, wkT, wvT, woT, lam, out_d, diff_d)
    nc.compile()
    return nc


def emit(tc, geneT, subT, wqT, wkT, wvT, woT, lam, out_d, diff_d):
    nc = tc.nc
    from contextlib import ExitStack
    with ExitStack() as ctx:
        consts = ctx.enter_context(tc.tile_pool(name="consts", bufs=1))
        kvp = ctx.enter_context(tc.tile_pool(name="kvp", bufs=1))
        gpool = ctx.enter_context(tc.tile_pool(name="gpool", bufs=3))
        qpool = ctx.enter_context(tc.tile_pool(name="qpool", bufs=2))
        ppool = ctx.enter_context(tc.tile_pool(name="ppool", bufs=3))
        dfpool = ctx.enter_context(tc.tile_pool(name="dfpool", bufs=3))
        dTpool = ctx.enter_context(tc.tile_pool(name="dTpool", bufs=3))
        sqpool = ctx.enter_context(tc.tile_pool(name="sqpool", bufs=3))
        nopool = ctx.enter_context(tc.tile_pool(name="nopool", bufs=3))
        ypool = ctx.enter_context(tc.tile_pool(name="ypool", bufs=3))
        dpool = ctx.enter_context(tc.tile_pool(name="dpool", bufs=64))
        # PSUM: 8 banks total = ps_s 2x[128,1024] (4) + ps_u 4x[128,512] (4)
        ps_s = ctx.enter_context(tc.tile_pool(name="ps_s", bufs=2, space="PSUM"))
        ps_u = ctx.enter_context(tc.tile_pool(name="ps_u", bufs=4, space="PSUM"))

        # ---- constants / input staging --------------------------------
        # subT + wk first: the kv-projection matmuls gate everything else,
        # so their inputs must land before the other 5 MB of weights.
        subT_r = subT.rearrange("(ki p) n -> p ki n", p=P)
        subT_sb = kvp.tile([P, KI, NS], F32R, tag="subT")
        w_sb = {}
        for name, wsrc in (("wk", wkT), ("wq", wqT), ("wv", wvT), ("wo", woT)):
            wtile = consts.tile([P, KI, E], F32R, tag=name)
            w_sb[name] = wtile
        # per-ki DMA slices so the first kv/q projections start as soon as
        # their first contraction tile lands instead of after the full 7 MB
        for ki in range(KI):
            nc.sync.dma_start(out=subT_sb[:, ki], in_=subT_r[:, ki])
            nc.sync.dma_start(out=w_sb["wk"][:, ki],
                              in_=wkT.rearrange("(ki p) e -> p ki e", p=P)[:, ki])
        for ki in range(KI):
            nc.sync.dma_start(out=w_sb["wv"][:, ki],
                              in_=wvT.rearrange("(ki p) e -> p ki e", p=P)[:, ki])
        for ki in range(KI):
            nc.sync.dma_start(out=w_sb["wq"][:, ki],
                              in_=wqT.rearrange("(ki p) e -> p ki e", p=P)[:, ki])
        nc.sync.dma_start(out=w_sb["wo"][:],
                          in_=woT.rearrange("(ki p) e -> p ki e", p=P))
        ident_f = consts.tile([P, P], F32, tag="ident_f")
        make_identity(nc, ident_f[:])
        ident = consts.tile([P, P], F32R, tag="ident")
        nc.vector.tensor_copy(ident[:], ident_f[:])
        lam_sb = consts.tile([P, 1], F32, tag="lam")
        nc.sync.dma_start(out=lam_sb[:], in_=lam)

        # ---- kv setup: kT [E, NS] and v [NS, E] ------------------------
        # per-block tiles so the first attention matmuls only wait on the
        # blocks they read (head-1 kT rows), not the whole kv projection
        kts = []
        for eo in range(EO):
            ktile = kvp.tile([P, NS], F32R, tag=f"kt{eo}")
            kts.append(ktile)
        vts = []
        for nb in range(NB):
            vtile = kvp.tile([P, E], F32R, tag=f"v{nb}")
            vts.append(vtile)

        for eo in range(EO):
            for nch in range(NS // 512):
                pk = ps_u.tile([P, 512], F32, tag="u")
                for ki in range(KI):
                    nc.tensor.matmul(
                        pk[:],
                        w_sb["wk"][:, ki, eo * P:(eo + 1) * P],
                        subT_sb[:, ki, nch * 512:(nch + 1) * 512],
                        start=(ki == 0), stop=(ki == KI - 1))
                nc.scalar.copy(kts[eo][:, nch * 512:(nch + 1) * 512], pk[:])

        for nb in range(NB):
            pv = ps_u.tile([P, 512], F32, tag="u")
            for ki in range(KI):
                nc.tensor.matmul(
                    pv[:],
                    subT_sb[:, ki, nb * P:(nb + 1) * P],
                    w_sb["wv"][:, ki, :],
                    start=(ki == 0), stop=(ki == KI - 1))
            nc.scalar.copy(vts[nb][:], pv[:])

        # ---- main loop over m-chunks ----------------------------------
        def produce_qT(c):
            gT = gpool.tile([P, KI, CHUNK], F32R, tag="gT")
            gsrc = geneT[:, c * CHUNK:(c + 1) * CHUNK].rearrange(
                "(ki p) m -> p ki m", p=P)
            for ki in range(KI):
                nc.sync.dma_start(out=gT[:, ki], in_=gsrc[:, ki])
            qT = qpool.tile([P, EO, CHUNK], F32R, tag="qT")
            for eo in range(EO):
                pq = ps_u.tile([P, 512], F32, tag="u")
                for ki in range(KI):
                    nc.tensor.matmul(
                        pq[:],
                        w_sb["wq"][:, ki, eo * P:(eo + 1) * P],
                        gT[:, ki, :],
                        start=(ki == 0), stop=(ki == KI - 1))
                nc.scalar.copy(qT[:, eo, :], pq[:])
            return qT

        def emit_S_exp(j_in_chunk, qT):
            """S matmuls + exp/accum for one m-subtile; returns softmax state."""
            j = j_in_chunk
            s1 = ps_s.tile([P, NS], F32, tag="s")
            s2 = ps_s.tile([P, NS], F32, tag="s")
            for hk in range(2):
                for nch in range(NS // 512):
                    nc.tensor.matmul(
                        s1[:, nch * 512:(nch + 1) * 512],
                        qT[:, hk, j * P:(j + 1) * P],
                        kts[hk][:, nch * 512:(nch + 1) * 512],
                        start=(hk == 0), stop=(hk == 1))
            p1 = ppool.tile([P, NS], F32, tag="p1")
            d1 = dpool.tile([P, 1], F32, tag="d")
            nc.scalar.activation(out=p1[:], in_=s1[:], func=AF.Exp,
                                 accum_out=d1[:])
            for hk in range(2):
                for nch in range(NS // 512):
                    nc.tensor.matmul(
                        s2[:, nch * 512:(nch + 1) * 512],
                        qT[:, 2 + hk, j * P:(j + 1) * P],
                        kts[2 + hk][:, nch * 512:(nch + 1) * 512],
                        start=(hk == 0), stop=(hk == 1))
            p2 = ppool.tile([P, NS], F32, tag="p2")
            d2 = dpool.tile([P, 1], F32, tag="d")
            nc.scalar.activation(out=p2[:], in_=s2[:], func=AF.Exp,
                                 accum_out=d2[:])
            return p1, d1, p2, d2

        def emit_softmax_tail(st, m0):
            """reciprocals + combine into diff; DMA diff out."""
            p1, d1, p2, d2 = st
            r1 = dpool.tile([P, 1], F32, tag="d")
            nc.vector.reciprocal(r1[:], d1[:])
            r2 = dpool.tile([P, 1], F32, tag="d")
            nc.vector.reciprocal(r2[:], d2[:])
            nc.vector.tensor_scalar(out=p2[:], in0=p2[:], scalar1=r2[:],
                                    scalar2=lam_sb[:], op0=ALU.mult,
                                    op1=ALU.mult)
            diff = dfpool.tile([P, NS], F32R, tag="diff")
            nc.vector.scalar_tensor_tensor(
                out=diff[:], in0=p1[:], scalar=r1[:], in1=p2[:],
                op0=ALU.mult, op1=ALU.subtract)
            nc.sync.dma_start(out=diff_d[m0:m0 + P, :], in_=diff[:])
            return diff

        def emit_transposes(diff):
            """diff -> diffT via PE transposes, copy to SBUF in quarters so
            the first PV matmuls can start while later blocks transpose."""
            dT = dTpool.tile([P, NS], F32R, tag="dT")
            for half in range(2):
                tt = ps_u.tile([P, 512], F32R, tag="u")
                for q in range(2):
                    for b in range(2):
                        nb = half * 4 + q * 2 + b
                        nc.tensor.transpose(
                            tt[:, (q * 2 + b) * P:(q * 2 + b + 1) * P],
                            diff[:, nb * P:(nb + 1) * P], ident[:])
                    nc.vector.tensor_copy(
                        dT[:, (half * 2 + q) * 256:(half * 2 + q + 1) * 256],
                        tt[:, q * 256:(q + 1) * 256])
            return dT

        def emit_back(dT, m0):
            """PV + RMS + Wo projection + store for one m-subtile."""
            po = ps_u.tile([P, E], F32, tag="u")
            for nb in range(NB):
                nc.tensor.matmul(
                    po[:],
                    dT[:, nb * P:(nb + 1) * P],
                    vts[nb][:],
                    start=(nb == 0), stop=(nb == NB - 1))

            # RMS statistic (squares + row-sum in one DVE pass)
            o_sb = nopool.tile([P, E], F32R, tag="no")
            nc.scalar.copy(o_sb[:], po[:])
            sq = sqpool.tile([P, E], F32, tag="sq")
            ssq = dpool.tile([P, 1], F32, tag="d")
            nc.vector.scalar_tensor_tensor(
                out=sq[:], in0=o_sb[:], scalar=1.0, in1=o_sb[:],
                op0=ALU.mult, op1=ALU.mult, accum_out=ssq[:])
            t_ssq = dpool.tile([P, 1], F32, tag="d")
            nc.vector.tensor_scalar(out=t_ssq[:], in0=ssq[:],
                                    scalar1=1.0 / E, scalar2=RMS_EPS,
                                    op0=ALU.mult, op1=ALU.add)
            # rstd = 1/sqrt(t): Quake bit-trick + 2 Newton iterations on DVE
            I32 = mybir.dt.int32
            ihalf = dpool.tile([P, 1], F32, tag="d")
            nc.vector.tensor_scalar(
                out=ihalf[:].bitcast(I32), in0=t_ssq[:].bitcast(I32),
                scalar1=1, scalar2=None, op0=ALU.arith_shift_right)
            inot = dpool.tile([P, 1], F32, tag="d")
            nc.vector.tensor_scalar(
                out=inot[:].bitcast(I32), in0=ihalf[:].bitcast(I32),
                scalar1=-1, scalar2=None, op0=ALU.bitwise_xor)
            yq = dpool.tile([P, 1], F32, tag="d")
            nc.vector.tensor_scalar(
                out=yq[:].bitcast(I32), in0=inot[:].bitcast(I32),
                scalar1=0x5f3759df + 1, scalar2=None, op0=ALU.add)
            rstd = yq
            for _ in range(2):
                y2 = dpool.tile([P, 1], F32, tag="d")
                nc.vector.tensor_mul(y2[:], rstd[:], rstd[:])
                w = dpool.tile([P, 1], F32, tag="d")
                nc.vector.tensor_scalar(out=w[:], in0=y2[:],
                                        scalar1=t_ssq[:], scalar2=-0.5,
                                        op0=ALU.mult, op1=ALU.mult)
                yn = dpool.tile([P, 1], F32, tag="d")
                nc.vector.scalar_tensor_tensor(
                    out=yn[:], in0=w[:], scalar=1.5, in1=rstd[:],
                    op0=ALU.add, op1=ALU.mult)
                rstd = yn

            return o_sb, rstd

        def emit_nt(o_sb):
            """transpose O (grouped with the diff transposes of the next
            subtile so PE transpose-mode switches stay rare)."""
            nt = ps_u.tile([P, E], F32R, tag="u")
            for eb in range(EO):
                nc.tensor.transpose(nt[:, eb * P:(eb + 1) * P],
                                    o_sb[:, eb * P:(eb + 1) * P], ident[:])
            noT = nopool.tile([P, E], F32R, tag="noT")
            nc.scalar.copy(noT[:, :256], nt[:, :256])
            nc.scalar.copy(noT[:, 256:], nt[:, 256:])
            return noT

        def emit_y(noT, rstd, m0):
            py = ps_u.tile([P, E], F32, tag="u")
            for eb in range(EO):
                nc.tensor.matmul(
                    py[:],
                    noT[:, eb * P:(eb + 1) * P],
                    w_sb["wo"][:, eb, :],
                    start=(eb == 0), stop=(eb == EO - 1))
            y = ypool.tile([P, E], F32, tag="y")
            nc.scalar.activation(out=y[:], in_=py[:], func=AF.Copy,
                                 scale=rstd[:])
            nc.sync.dma_start(out=out_d[m0:m0 + P, :], in_=y[:])

        # Software-pipelined emission, three-deep skew. Per step the PE
        # stream is: S(i) -> nt(i-3) -> tt(i-1) -> PV(i-2) -> y(i-3).
        # Transposes of consecutive stages sit adjacent (fewer PE
        # transpose-mode switches) and every cross-engine hop -- including
        # the DVE diffT-cast feeding PV -- has a full subtile of slack.
        NTOT = NCH * JT
        cur_qT, next_qT = produce_qT(0), None
        diffs, dTs, pvs, noTs = {}, {}, {}, {}
        for idx in range(NTOT + 3):
            c, j = divmod(idx, JT)
            if idx < NTOT:
                if j == 0 and c > 0:
                    cur_qT = next_qT
                st = emit_S_exp(j, cur_qT)
            if idx - 3 >= 0:
                noTs[idx - 3] = emit_nt(pvs[idx - 3][0])
            if idx - 1 >= 0 and idx - 1 < NTOT:
                dTs[idx - 1] = emit_transposes(diffs.pop(idx - 1))
            if idx < NTOT:
                diffs[idx] = emit_softmax_tail(st, idx * P)
            if idx - 2 >= 0 and idx - 2 < NTOT:
                pvs[idx - 2] = emit_back(dTs.pop(idx - 2), (idx - 2) * P)
            if idx - 3 >= 0:
                emit_y(noTs.pop(idx - 3), pvs.pop(idx - 3)[1], (idx - 3) * P)
            if idx < NTOT and j == 0 and c + 1 < NCH:
                next_qT = produce_qT(c + 1)


# ---------------------------------------------------------------------------
_NC = None


def get_nc():
    global _NC
    if _NC is None:
        _NC = build_kernel()
    return _NC


def stage_inputs(gene, substructure, Wq, Wk, Wv, Wo,
                 lambda_q1, lambda_k1, lambda_q2, lambda_k2, rms_weight):
    gene = np.asarray(gene, np.float32)
    substructure = np.asarray(substructure, np.float32)
    scaling = H ** -0.5
    lam_full = (math.exp(float(np.sum(np.asarray(lambda_q1, np.float64) *
                                      np.asarray(lambda_k1, np.float64))))
                - math.exp(float(np.sum(np.asarray(lambda_q2, np.float64) *
                                        np.asarray(lambda_k2, np.float64))))
                + LAMBDA_INIT)
    wqT = np.ascontiguousarray(np.asarray(Wq, np.float32).T * scaling)
    wkT = np.ascontiguousarray(np.asarray(Wk, np.float32).T)
    wvT = np.ascontiguousarray(np.asarray(Wv, np.float32).T)
    woT = np.ascontiguousarray(
        np.asarray(rms_weight, np.float32)[:, None]
        * np.asarray(Wo, np.float32).T * (1.0 - LAMBDA_INIT))
    lam_tile = np.full((P, 1), lam_full, np.float32)
    in_maps = []
    for b in range(N_CORES):
        in_maps.append({
            "geneT": np.ascontiguousarray(gene[b].T),
            "subT": np.ascontiguousarray(substructure[b].T),
            "wqT": wqT, "wkT": wkT, "wvT": wvT, "woT": woT,
            "lam": lam_tile,
        })
    return in_maps


def run(in_maps, trace=False, **kw):
    nc = get_nc()
    last_err = None
    for attempt in range(3):
        try:
            return bass_utils.run_bass_kernel_spmd(
                nc, in_maps, core_ids=list(range(N_CORES)), trace=trace, **kw)
        except Exception as e:  # transient device errors on first touch
            last_err = e
    raise last_err


def kernel(**inputs):
    in_maps = stage_inputs(**inputs)
    res = run(in_maps, trace=False)
    out = np.stack([res.results[b]["out"] for b in range(N_CORES)])
    diff = np.stack([res.results[b]["diff"] for b in range(N_CORES)])
    return out, diff
